# revision 2
# baseline (speedup 1.0000x reference)
"""CSSM TinyViT block on 8 TRN2 NeuronCores.

Strategy
--------
Data-parallel over batch: B=16 -> 2 samples (2048 tokens) per core.
Matmuls run in bf16 (error budget 2e-2; measured ~1e-3).

Layouts: LayerNorm stats/apply run token-major ([128 tok, 384 c], free-dim
reduction via bn_stats).  All channel-mixing matmuls run channel-major
([128 c, tok] tiles, weights stationary).  PE transposes move between the
two.  The final MLP matmul consumes the hidden activations as the
*stationary* operand, producing token-major output directly.

Key optimizations over the naive schedule:
- rstd = rsqrt(var+eps) is computed on the vector engine with a Newton
  iteration (poly init + 3 steps) instead of scalar-engine Sqrt: the sqrt
  activation lives in a different hardware table set than tanh/gelu, and
  interleaving them forces a ~2.7us table reload each time (13 reloads =
  ~35us serial scalar time).  With Newton-rsqrt only the gelu_apprx_tanh
  set is ever loaded.
- The T=8 gated scan  h <- g*(h@A) + m2  is factorized with the doubling
  identity sum_{k=0}^{7} B^k = (I+B)(I+B^2)(I+B^4), B v = g*(A v):
  7 gated applications + 3 adds instead of 7 muls + 7 adds.
- Each scan application accumulates all 3 output channel tiles into one
  contiguous [128, 3, 512] PSUM tile and applies the gate with a single
  big tensor_tensor, amortizing the DVE PSUM-read overhead.
- Elementwise work is spread across engines: PSUM->SBUF transpose copies
  and u+bu biasing on the scalar engine (activation with per-partition
  bias), LN applies / m2 / one scan add on gpsimd, the rest on vector.

Host-side preprocessing folds the LayerNorm scale/bias into the
downstream weight matrices, so the device only ever normalizes.

The scan runs with the sign-flipped state h' = -h so that m2' = (g-1)*ub
needs no extra negation: h' = sum B^k m2', and x + h = x - h'.
"""
import json
import types

import numpy as np

import concourse.bass as bass
import concourse.mybir as mybir
from concourse.tile import TileContext
from concourse.bass_utils import run_bass_kernel_spmd

F32 = mybir.dt.float32
F32R = mybir.dt.float32r
BF16 = mybir.dt.bfloat16
AF = mybir.ActivationFunctionType
OP = mybir.AluOpType

import os
DT_MM = F32R if os.environ.get("KERNEL_MM_DT", "bf16") == "f32r" else BF16

B, H, W, C, T = 16, 32, 32, 384, 8
HID = 4 * C
EPS = 1e-6
NCORES = 8
BSH = B // NCORES              # samples per core
NTOK = BSH * H * W             # 2048 tokens per core
GTOK = 512                     # tokens per group
NG = NTOK // GTOK              # 4 groups
TPG = GTOK // 128              # 4 token-tiles per group
KT = C // 128                  # 3 channel tiles
MH = HID // 128                # 12 hidden tiles

# minimax-ish quadratic init for rsqrt(v) on v in [0.25, 4]; 3 Newton steps
RS_C2, RS_C1, RS_C0 = 0.08414093509693645, -0.5905266440563495, 1.563593294299663


# ---------------------------------------------------------------- bir fix --
# This container's walrus rejects instructions whose sync-wait list exceeds
# the opcode's wait slots (an SP Drain has none free).  Move excess waits
# onto EventSemaphore instructions inserted before the instruction on the
# same engine queue; waits still happen-before, so semantics are unchanged.
_WAIT_LIMITS = {"Drain": 0}
_WAIT_DEFAULT = 1


def _fix_bir_json(bj: bytes) -> bytes:
    bir = json.loads(bj)
    counter = [0]

    def fix_blocks(blocks):
        for b in blocks:
            insts = b.get("instructions")
            if insts:
                new = []
                for inst in insts:
                    si = inst.get("sync_info")
                    waits = (si or {}).get("on_wait") or []
                    limit = _WAIT_LIMITS.get(inst.get("opcode"), _WAIT_DEFAULT)
                    if len(waits) > limit:
                        n_extra = len(waits) - limit
                        extra, keep = waits[:n_extra], waits[n_extra:]
                        for wv in extra:
                            counter[0] += 1
                            new.append({
                                "name": f"I-wfix-{counter[0]}",
                                "opcode": "EventSemaphore",
                                "engine": inst["engine"],
                                "ins": [],
                                "outs": [],
                                "sync_info": {"on_update": [], "on_wait": [wv]},
                                "debug": inst.get("debug", 0),
                            })
                        si["on_wait"] = keep
                    new.append(inst)
                b["instructions"] = new
            fix_blocks(b.get("blocks") or [])

    for fn in bir.get("functions", []):
        fix_blocks(fn.get("blocks") or [])
    return json.dumps(bir).encode()


def _patch_nc(nc):
    orig = nc.to_json_bytes

    def to_json_bytes(self):
        return _fix_bir_json(orig())

    nc.to_json_bytes = types.MethodType(to_json_bytes, nc)
    return nc


# ----------------------------------------------------------- device build --
def build_nc(repeat=1):
    nc = bass.Bass()

    x_in = nc.declare_dram_parameter("x", [NTOK, C], F32, isOutput=False)
    wu_d = nc.declare_dram_parameter("wu", [C, C], DT_MM, isOutput=False)
    wg_d = nc.declare_dram_parameter("wg", [C, C], DT_MM, isOutput=False)
    a_d = nc.declare_dram_parameter("a", [C, C], DT_MM, isOutput=False)
    w1_d = nc.declare_dram_parameter("w1", [C, HID], DT_MM, isOutput=False)
    w2_d = nc.declare_dram_parameter("w2", [HID, C], DT_MM, isOutput=False)
    bias_d = nc.declare_dram_parameter("bias", [128, 2 * KT + MH], F32,
                                       isOutput=False)
    b2_d = nc.declare_dram_parameter("b2", [1, C], DT_MM, isOutput=False)
    eye_d = nc.declare_dram_parameter("eye", [128, 128], DT_MM, isOutput=False)
    ones_d = nc.declare_dram_parameter("ones", [1, 128], DT_MM, isOutput=False)
    out_d = nc.declare_dram_parameter("out", [NTOK, C], F32, isOutput=True)

    with TileContext(nc) as tc:
        with (
            tc.tile_pool(name="wp", bufs=1) as wp,
            tc.tile_pool(name="gp", bufs=2) as gp,
            tc.tile_pool(name="hidp", bufs=1) as hidp,
            tc.tile_pool(name="hp", bufs=4) as hp,
            tc.tile_pool(name="tp", bufs=3) as tp,
            tc.tile_pool(name="sp", bufs=4) as sp,
            tc.tile_pool(name="ps", bufs=2, space="PSUM") as ps,
            tc.tile_pool(name="pst", bufs=2, space="PSUM") as pst,
        ):
            # ---- weights / constants (loaded once) ----
            wu_t = [wp.tile([128, C], DT_MM, tag=f"wu{k}", name=f"wu{k}") for k in range(KT)]
            wg_t = [wp.tile([128, C], DT_MM, tag=f"wg{k}", name=f"wg{k}") for k in range(KT)]
            a_t = [wp.tile([128, C], DT_MM, tag=f"a{k}", name=f"a{k}") for k in range(KT)]
            w1_t = [wp.tile([128, HID], DT_MM, tag=f"w1{k}", name=f"w1{k}") for k in range(KT)]
            w2_t = [wp.tile([128, C], DT_MM, tag=f"w2{k}", name=f"w2{k}") for k in range(MH)]
            bias_t = wp.tile([128, 2 * KT + MH], F32, tag="bias")
            b2_t = wp.tile([1, C], DT_MM, tag="b2")
            eye_t = wp.tile([128, 128], DT_MM, tag="eye")
            ones_t = wp.tile([1, 128], DT_MM, tag="ones")
            # order matters: the SP DMA queue drains in program order, and
            # the first PE work needs eye (transposes) then wu/wg (phase B).
            nc.sync.dma_start(out=eye_t, in_=eye_d[:, :])
            nc.sync.dma_start(out=bias_t, in_=bias_d[:, :])

            def load_mid_weights():
                for k in range(KT):
                    s = slice(k * 128, (k + 1) * 128)
                    nc.sync.dma_start(out=wu_t[k], in_=wu_d[s, :])
                    nc.sync.dma_start(out=wg_t[k], in_=wg_d[s, :])
                for k in range(KT):
                    s = slice(k * 128, (k + 1) * 128)
                    nc.sync.dma_start(out=a_t[k], in_=a_d[s, :])
            def load_late_weights():
                # w1/w2 are first needed ~40us in; issuing them after the
                # first pair's x loads keeps the SP queue from delaying the
                # critical path.
                for k in range(KT):
                    s = slice(k * 128, (k + 1) * 128)
                    nc.sync.dma_start(out=w1_t[k], in_=w1_d[s, :])
                for k in range(MH):
                    nc.sync.dma_start(out=w2_t[k],
                                      in_=w2_d[k * 128:(k + 1) * 128, :])
                nc.sync.dma_start(out=b2_t, in_=b2_d[:, :])
                nc.sync.dma_start(out=ones_t, in_=ones_d[:, :])

            def newton_rstd(mvb, tag):
                """rstd[:, i] = rsqrt(var[:, i] + eps) for 4 batched tiles.

                Quadratic init + 3 Newton steps, all on the vector engine
                (no scalar-engine Sqrt -> no activation-table thrash).
                """
                v = mvb[:, :, 1:2].rearrange("p a b -> p (a b)")
                y = sp.tile([128, TPG], F32, tag=f"nsy{tag}", name=f"nsy{tag}")
                t = sp.tile([128, TPG], F32, tag=f"nst{tag}", name=f"nst{tag}")
                nc.vector.tensor_scalar(out=t, in0=v, scalar1=RS_C2,
                                        scalar2=RS_C1, op0=OP.mult, op1=OP.add)
                nc.vector.tensor_mul(out=y, in0=t, in1=v)
                nc.vector.tensor_scalar(out=y, in0=y, scalar1=RS_C0,
                                        scalar2=None, op0=OP.add)
                for _ in range(3):
                    nc.vector.tensor_mul(out=t, in0=y, in1=y)
                    nc.vector.scalar_tensor_tensor(out=t, in0=v, scalar=EPS,
                                                   in1=t, op0=OP.add,
                                                   op1=OP.mult)
                    nc.vector.tensor_scalar(out=t, in0=t, scalar1=-0.5,
                                            scalar2=1.5, op0=OP.mult,
                                            op1=OP.add)
                    nc.vector.tensor_mul(out=y, in0=y, in1=t)
                return y

            def phase_a(grp):
                """load + norm1 + transpose -> channel-major xn"""
                st = {}
                st["x_tm"] = x_tm = gp.tile([128, TPG, C], F32, tag="x_tm",
                                            name=f"x_tm{grp}", bufs=3)
                st["xn_cm"] = xn_cm = gp.tile([128, KT, GTOK], DT_MM,
                                              tag="xn_cm", name=f"xn_cm{grp}")
                mvb = sp.tile([128, TPG, 2], F32, tag="mvb", name=f"mvb{grp}")
                for it in range(TPG):
                    row0 = (grp * TPG + it) * 128
                    nc.sync.dma_start(out=x_tm[:, it, :],
                                      in_=x_in[row0:row0 + 128, :])
                    mv6 = sp.tile([128, 6], F32, tag="mv6")
                    nc.vector.bn_stats(out=mv6, in_=x_tm[:, it, :])
                    nc.vector.bn_aggr(out=mvb[:, it, :], in_=mv6)
                rstd = newton_rstd(mvb, "a")
                for it in range(TPG):
                    xn = tp.tile([128, C], DT_MM, tag="xn", bufs=2)
                    nc.vector.tensor_scalar(out=xn, in0=x_tm[:, it, :],
                                            scalar1=mvb[:, it, 0:1],
                                            scalar2=rstd[:, it:it + 1],
                                            op0=OP.subtract, op1=OP.mult)
                    pt = pst.tile([128, KT, 128], DT_MM, tag="pt")
                    for c in range(KT):
                        nc.tensor.transpose(pt[:, c, :],
                                            xn[:, c * 128:(c + 1) * 128],
                                            eye_t)
                    nc.scalar.copy(
                        out=xn_cm[:, :, it * 128:(it + 1) * 128], in_=pt)
                return st

            def phase_b(grp, st):
                """u/g projections, gate, m2"""
                xn_cm = st["xn_cm"]
                st["g"] = g_t = gp.tile([128, KT, GTOK], F32, tag="g",
                                        name=f"g{grp}")
                st["m2"] = m2_t = gp.tile([128, KT, GTOK], DT_MM, tag="m2",
                                          name=f"m2{grp}")
                psu = ps.tile([128, KT, GTOK], F32, tag="big", name=f"psu{grp}")
                psg = ps.tile([128, KT, GTOK], F32, tag="big", name=f"psg{grp}")
                for m in range(KT):
                    msl = slice(m * 128, (m + 1) * 128)
                    for k in range(KT):
                        nc.tensor.matmul(psu[:, m, :], wu_t[k][:, msl],
                                         xn_cm[:, k, :],
                                         start=(k == 0), stop=(k == KT - 1))
                    for k in range(KT):
                        nc.tensor.matmul(psg[:, m, :], wg_t[k][:, msl],
                                         xn_cm[:, k, :],
                                         start=(k == 0), stop=(k == KT - 1))
                for m in range(KT):
                    th = tp.tile([128, GTOK], F32, tag="th", bufs=2)
                    nc.scalar.activation(out=th, in_=psg[:, m, :], func=AF.Tanh,
                                         bias=bias_t[:, KT + m:KT + m + 1],
                                         scale=0.5)
                    nc.gpsimd.tensor_scalar(out=g_t[:, m, :], in0=th,
                                            scalar1=0.5, scalar2=0.5,
                                            op0=OP.mult, op1=OP.add)
                    ub = tp.tile([128, GTOK], DT_MM, tag="ub", bufs=2)
                    nc.scalar.activation(out=ub, in_=psu[:, m, :],
                                         func=AF.Identity,
                                         bias=bias_t[:, m:m + 1])
                    # m2 = (g - 1) * (u + bu)   (= -(1-g)*u_b)
                    nc.vector.scalar_tensor_tensor(
                        out=m2_t[:, m, :], in0=g_t[:, m, :], scalar=1.0,
                        in1=ub, op0=OP.subtract, op1=OP.mult)

            def scan_app(grp, st, src, tag):
                """dst = g * (A^T src), one gated application of B."""
                g_t = st["g"]
                dst = hp.tile([128, KT, GTOK], DT_MM, tag=tag,
                              name=f"{tag}{grp}")
                psb = ps.tile([128, KT, GTOK], F32, tag="big",
                              name=f"psb{grp}")
                for m in range(KT):
                    msl = slice(m * 128, (m + 1) * 128)
                    for k in range(KT):
                        nc.tensor.matmul(psb[:, m, :], a_t[k][:, msl],
                                         src[:, k, :],
                                         start=(k == 0), stop=(k == KT - 1))
                nc.vector.tensor_mul(out=dst, in0=g_t, in1=psb)
                return dst

            def residual1(grp, st):
                """x2 = x + h = x - h'  (token-major)"""
                h_t, x_tm = st["h"], st["x_tm"]
                st["x2_tm"] = x2_tm = gp.tile([128, TPG, C], F32, tag="x2_tm",
                                              name=f"x2_tm{grp}")
                for it in range(TPG):
                    pt = pst.tile([128, KT, 128], DT_MM, tag="pt")
                    for c in range(KT):
                        nc.tensor.transpose(
                            pt[:, c, :],
                            h_t[:, c, it * 128:(it + 1) * 128], eye_t)
                    nc.vector.tensor_sub(
                        out=x2_tm[:, it, :].rearrange("p (c q) -> p c q", c=KT),
                        in0=x_tm[:, it, :].rearrange("p (c q) -> p c q", c=KT),
                        in1=pt)

            def norm2(grp, st):
                x2_tm = st["x2_tm"]
                st["xn2_cm"] = xn2_cm = gp.tile([128, KT, GTOK], DT_MM,
                                                tag="xn2_cm",
                                                name=f"xn2_cm{grp}")
                mvb = sp.tile([128, TPG, 2], F32, tag="mvb2", name=f"mvb2{grp}")
                for it in range(TPG):
                    mv6 = sp.tile([128, 6], F32, tag="mv6")
                    nc.vector.bn_stats(out=mv6, in_=x2_tm[:, it, :])
                    nc.vector.bn_aggr(out=mvb[:, it, :], in_=mv6)
                rstd = newton_rstd(mvb, "b")
                for it in range(TPG):
                    xn2 = tp.tile([128, C], DT_MM, tag="xn", bufs=2)
                    nc.gpsimd.tensor_scalar(out=xn2, in0=x2_tm[:, it, :],
                                            scalar1=mvb[:, it, 0:1],
                                            scalar2=rstd[:, it:it + 1],
                                            op0=OP.subtract, op1=OP.mult)
                    pt = pst.tile([128, KT, 128], DT_MM, tag="pt")
                    for c in range(KT):
                        nc.tensor.transpose(pt[:, c, :],
                                            xn2[:, c * 128:(c + 1) * 128],
                                            eye_t)
                    nc.scalar.copy(
                        out=xn2_cm[:, :, it * 128:(it + 1) * 128], in_=pt)

            def mlp(grp, st):
                xn2_cm, x2_tm = st["xn2_cm"], st["x2_tm"]
                hid_t = hidp.tile([128, MH, GTOK], DT_MM, tag="hid",
                                  name=f"hid{grp}")
                for b in range(MH // KT):
                    psm = ps.tile([128, KT, GTOK], F32, tag="big",
                                  name=f"psm{grp}")
                    for m3 in range(KT):
                        mh = b * KT + m3
                        msl = slice(mh * 128, (mh + 1) * 128)
                        for k in range(KT):
                            nc.tensor.matmul(psm[:, m3, :], w1_t[k][:, msl],
                                             xn2_cm[:, k, :],
                                             start=(k == 0), stop=(k == KT - 1))
                    for m3 in range(KT):
                        mh = b * KT + m3
                        nc.scalar.activation(
                            out=hid_t[:, mh, :], in_=psm[:, m3, :],
                            func=AF.Gelu_apprx_tanh,
                            bias=bias_t[:, 2 * KT + mh:2 * KT + mh + 1],
                            scale=1.0)
                # second matmul: hidden is the stationary operand -> output
                # lands token-major; fold mlp_b2 in via a K=1 matmul.
                for it in range(TPG):
                    tsl = slice(it * 128, (it + 1) * 128)
                    pso = pst.tile([128, C], F32, tag="pt", name=f"pso{grp}")
                    for mh in range(MH):
                        nc.tensor.matmul(pso, hid_t[:, mh, tsl], w2_t[mh],
                                         start=(mh == 0), stop=False)
                    nc.tensor.matmul(pso, ones_t, b2_t,
                                     start=False, stop=True)
                    nc.vector.tensor_add(out=x2_tm[:, it, :],
                                         in0=x2_tm[:, it, :], in1=pso)
                    row0 = (grp * TPG + it) * 128
                    nc.sync.dma_start(out=out_d[row0:row0 + 128, :],
                                      in_=x2_tm[:, it, :])

            def scan(states, g0, g1):
                """h' = (I+B)(I+B^2)(I+B^4) m2', pairwise interleaved."""
                s0, s1 = states[g0], states[g1]
                cur = {g0: s0["m2"], g1: s1["m2"]}
                # b4 = B^4 m2
                for i in range(4):
                    cur[g0] = scan_app(g0, s0, cur[g0], "h")
                    cur[g1] = scan_app(g1, s1, cur[g1], "h")
                # r1 = m2 + b4
                r1 = {}
                for g, s in ((g0, s0), (g1, s1)):
                    r1[g] = hp.tile([128, KT, GTOK], DT_MM, tag="w",
                                    name=f"r1_{g}")
                    nc.vector.tensor_add(out=r1[g], in0=s["m2"], in1=cur[g])
                cur = dict(r1)
                for i in range(2):
                    cur[g0] = scan_app(g0, s0, cur[g0], "h")
                    cur[g1] = scan_app(g1, s1, cur[g1], "h")
                # r2 = r1 + c2
                r2 = {}
                for g in (g0, g1):
                    r2[g] = hp.tile([128, KT, GTOK], DT_MM, tag="w",
                                    name=f"r2_{g}")
                    nc.vector.tensor_add(out=r2[g], in0=r1[g], in1=cur[g])
                # d1 = B r2 ; h = r2 + d1
                d1 = {}
                d1[g0] = scan_app(g0, s0, r2[g0], "h")
                d1[g1] = scan_app(g1, s1, r2[g1], "h")
                for g, s in ((g0, s0), (g1, s1)):
                    h = hp.tile([128, KT, GTOK], DT_MM, tag="w", name=f"h_{g}")
                    nc.gpsimd.tensor_add(out=h, in0=r2[g], in1=d1[g])
                    s["h"] = h

            # Pairwise interleave groups so the PE fills one group's
            # scan/norm dependency stalls with the other group's matmuls;
            # additionally pipeline the next pair's phase A into the
            # current pair's norm2/MLP window.
            npair = (NG // 2) * repeat
            states = {}
            for pair_i in range(npair):
                pair = pair_i % (NG // 2)
                g0, g1 = 2 * pair, 2 * pair + 1
                if pair_i == 0:
                    states[g0] = phase_a(g0)
                    states[g1] = phase_a(g1)
                    load_mid_weights()
                s0, s1 = states[g0], states[g1]
                phase_b(g0, s0)
                phase_b(g1, s1)
                if pair_i == 0:
                    load_late_weights()
                scan(states, g0, g1)
                residual1(g0, s0)
                residual1(g1, s1)
                norm2(g0, s0)
                norm2(g1, s1)
                if pair_i + 1 < npair:
                    nx = 2 * ((pair_i + 1) % (NG // 2))
                    states[nx] = phase_a(nx)
                    states[nx + 1] = phase_a(nx + 1)
                mlp(g0, s0)
                mlp(g1, s1)
    return nc


_NC_CACHE = {}


def _get_nc():
    if "nc" not in _NC_CACHE:
        _NC_CACHE["nc"] = _patch_nc(build_nc())
    return _NC_CACHE["nc"]


# ---------------------------------------------------------------- kernel --
def kernel(x, norm1_scale, norm1_bias, Wu, bu, Wg, bg, A,
           norm2_scale, norm2_bias, mlp_w1, mlp_b1, mlp_w2, mlp_b2,
           _return_raw=False):
    f = np.float32
    x = np.asarray(x, f)
    norm1_scale = np.asarray(norm1_scale, f)
    norm1_bias = np.asarray(norm1_bias, f)
    Wu, bu = np.asarray(Wu, f), np.asarray(bu, f)
    Wg, bg = np.asarray(Wg, f), np.asarray(bg, f)
    A = np.asarray(A, f)
    norm2_scale = np.asarray(norm2_scale, f)
    norm2_bias = np.asarray(norm2_bias, f)
    mlp_w1, mlp_b1 = np.asarray(mlp_w1, f), np.asarray(mlp_b1, f)
    mlp_w2, mlp_b2 = np.asarray(mlp_w2, f), np.asarray(mlp_b2, f)

    # fold LN affine into downstream weights
    wu = np.ascontiguousarray(norm1_scale[:, None] * Wu)
    bu_f = bu + norm1_bias @ Wu
    wg = np.ascontiguousarray(norm1_scale[:, None] * Wg)
    bg_f = bg + norm1_bias @ Wg
    w1 = np.ascontiguousarray(norm2_scale[:, None] * mlp_w1)
    b1_f = mlp_b1 + norm2_bias @ mlp_w1

    # bias pack: [128, KT + KT + MH] columns = bu tiles, bg tiles, b1 tiles
    bias = np.empty((128, 2 * KT + MH), f)
    for m in range(KT):
        bias[:, m] = bu_f[m * 128:(m + 1) * 128]
        bias[:, KT + m] = 0.5 * bg_f[m * 128:(m + 1) * 128]
    for m in range(MH):
        bias[:, 2 * KT + m] = b1_f[m * 128:(m + 1) * 128]

    eye = np.eye(128, dtype=f)
    b2row = np.ascontiguousarray(mlp_b2[None, :])
    w2 = mlp_w2
    if DT_MM == BF16:
        import ml_dtypes
        bf = ml_dtypes.bfloat16
        wu, wg, w1 = wu.astype(bf), wg.astype(bf), w1.astype(bf)
        A = A.astype(bf)
        w2 = np.asarray(mlp_w2, f).astype(bf)
        b2row = b2row.astype(bf)
        eye = eye.astype(bf)
        ones = np.ones((1, 128), f).astype(bf)
    else:
        ones = np.ones((1, 128), f)

    xs = x.reshape(NCORES, NTOK, C)
    in_maps = [{
        "x": np.ascontiguousarray(xs[i]),
        "wu": wu, "wg": wg, "a": A, "w1": w1, "w2": w2,
        "bias": bias, "b2": b2row, "eye": eye, "ones": ones,
    } for i in range(NCORES)]

    res = run_bass_kernel_spmd(_get_nc(), in_maps, list(range(NCORES)))
    if _return_raw:
        return res
    out = np.concatenate([res.results[i]["out"] for i in range(NCORES)], axis=0)
    return out.reshape(B, H, W, C).astype(np.float32)


# revision 4
# speedup vs baseline: 1.4094x; 1.4094x over previous
"""CSSM TinyViT block on 8 TRN2 NeuronCores.

Strategy
--------
Data-parallel over batch: B=16 -> 2 samples (2048 tokens) per core.
Matmuls run in bf16 (error budget 2e-2; measured ~1e-3).

Layouts: LayerNorm stats/apply run token-major ([128 tok, 384 c], free-dim
reduction via bn_stats).  All channel-mixing matmuls run channel-major
([128 c, tok] tiles, weights stationary).  PE transposes move between the
two.  The final MLP matmul consumes the hidden activations as the
*stationary* operand, producing token-major output directly.

Key optimizations over the naive schedule:
- rstd = rsqrt(var+eps) is computed on the vector engine with a Newton
  iteration (poly init + 3 steps) instead of scalar-engine Sqrt: the sqrt
  activation lives in a different hardware table set than tanh/gelu, and
  interleaving them forces a ~2.7us table reload each time (13 reloads =
  ~35us serial scalar time).  With Newton-rsqrt only the gelu_apprx_tanh
  set is ever loaded.
- The T=8 gated scan  h <- g*(h@A) + m2  is factorized with the doubling
  identity sum_{k=0}^{7} B^k = (I+B)(I+B^2)(I+B^4), B v = g*(A v):
  7 gated applications + 3 adds instead of 7 muls + 7 adds.
- Each scan application accumulates all 3 output channel tiles into one
  contiguous [128, 3, 512] PSUM tile and applies the gate with a single
  big tensor_tensor, amortizing the DVE PSUM-read overhead.
- Elementwise work is spread across engines: PSUM->SBUF transpose copies
  and u+bu biasing on the scalar engine (activation with per-partition
  bias), LN applies / m2 / one scan add on gpsimd, the rest on vector.

Host-side preprocessing folds the LayerNorm scale/bias into the
downstream weight matrices, so the device only ever normalizes.

The scan runs with the sign-flipped state h' = -h so that m2' = (g-1)*ub
needs no extra negation: h' = sum B^k m2', and x + h = x - h'.
"""
import json
import types

import numpy as np

import concourse.bass as bass
import concourse.mybir as mybir
from concourse.tile import TileContext
from concourse.bass_utils import run_bass_kernel_spmd

F32 = mybir.dt.float32
F32R = mybir.dt.float32r
BF16 = mybir.dt.bfloat16
AF = mybir.ActivationFunctionType
OP = mybir.AluOpType

import os
DT_MM = F32R if os.environ.get("KERNEL_MM_DT", "bf16") == "f32r" else BF16

B, H, W, C, T = 16, 32, 32, 384, 8
HID = 4 * C
EPS = 1e-6
NCORES = 8
BSH = B // NCORES              # samples per core
NTOK = BSH * H * W             # 2048 tokens per core
GTOK = 512                     # tokens per group
NG = NTOK // GTOK              # 4 groups
TPG = GTOK // 128              # 4 token-tiles per group
KT = C // 128                  # 3 channel tiles
MH = HID // 128                # 12 hidden tiles

# minimax-ish quadratic init for rsqrt(v) on v in [0.25, 4]; 3 Newton steps
RS_C2, RS_C1, RS_C0 = 0.08414093509693645, -0.5905266440563495, 1.563593294299663


# ---------------------------------------------------------------- bir fix --
# This container's walrus rejects instructions whose sync-wait list exceeds
# the opcode's wait slots (an SP Drain has none free).  Move excess waits
# onto EventSemaphore instructions inserted before the instruction on the
# same engine queue; waits still happen-before, so semantics are unchanged.
_WAIT_LIMITS = {"Drain": 0}
_WAIT_DEFAULT = 1


def _fix_bir_json(bj: bytes) -> bytes:
    bir = json.loads(bj)
    counter = [0]

    def fix_blocks(blocks):
        for b in blocks:
            insts = b.get("instructions")
            if insts:
                new = []
                for inst in insts:
                    si = inst.get("sync_info")
                    waits = (si or {}).get("on_wait") or []
                    limit = _WAIT_LIMITS.get(inst.get("opcode"), _WAIT_DEFAULT)
                    if len(waits) > limit:
                        n_extra = len(waits) - limit
                        extra, keep = waits[:n_extra], waits[n_extra:]
                        for wv in extra:
                            counter[0] += 1
                            new.append({
                                "name": f"I-wfix-{counter[0]}",
                                "opcode": "EventSemaphore",
                                "engine": inst["engine"],
                                "ins": [],
                                "outs": [],
                                "sync_info": {"on_update": [], "on_wait": [wv]},
                                "debug": inst.get("debug", 0),
                            })
                        si["on_wait"] = keep
                    new.append(inst)
                b["instructions"] = new
            fix_blocks(b.get("blocks") or [])

    for fn in bir.get("functions", []):
        fix_blocks(fn.get("blocks") or [])
    return json.dumps(bir).encode()


def _patch_nc(nc):
    orig = nc.to_json_bytes

    def to_json_bytes(self):
        return _fix_bir_json(orig())

    nc.to_json_bytes = types.MethodType(to_json_bytes, nc)
    return nc


# ----------------------------------------------------------- device build --
def build_nc(repeat=1):
    nc = bass.Bass()

    x_in = nc.declare_dram_parameter("x", [NTOK, C], F32, isOutput=False)
    wu_d = nc.declare_dram_parameter("wu", [C, C], DT_MM, isOutput=False)
    wg_d = nc.declare_dram_parameter("wg", [C, C], DT_MM, isOutput=False)
    a_d = nc.declare_dram_parameter("a", [C, C], DT_MM, isOutput=False)
    w1_d = nc.declare_dram_parameter("w1", [C, HID], DT_MM, isOutput=False)
    w2_d = nc.declare_dram_parameter("w2", [HID, C], DT_MM, isOutput=False)
    bias_d = nc.declare_dram_parameter("bias", [128, 2 * KT + MH], F32,
                                       isOutput=False)
    b2_d = nc.declare_dram_parameter("b2", [1, C], DT_MM, isOutput=False)
    eye_d = nc.declare_dram_parameter("eye", [128, 128], DT_MM, isOutput=False)
    ones_d = nc.declare_dram_parameter("ones", [1, 128], DT_MM, isOutput=False)
    out_d = nc.declare_dram_parameter("out", [NTOK, C], F32, isOutput=True)

    with TileContext(nc) as tc:
        with (
            tc.tile_pool(name="wp", bufs=1) as wp,
            tc.tile_pool(name="gp", bufs=2) as gp,
            tc.tile_pool(name="hidp", bufs=1) as hidp,
            tc.tile_pool(name="hp", bufs=4) as hp,
            tc.tile_pool(name="tp", bufs=3) as tp,
            tc.tile_pool(name="sp", bufs=4) as sp,
            tc.tile_pool(name="ps", bufs=2, space="PSUM") as ps,
            tc.tile_pool(name="pst", bufs=2, space="PSUM") as pst,
        ):
            # ---- weights / constants (loaded once) ----
            wu_t = [wp.tile([128, C], DT_MM, tag=f"wu{k}", name=f"wu{k}") for k in range(KT)]
            wg_t = [wp.tile([128, C], DT_MM, tag=f"wg{k}", name=f"wg{k}") for k in range(KT)]
            a_t = [wp.tile([128, C], DT_MM, tag=f"a{k}", name=f"a{k}") for k in range(KT)]
            w1_t = [wp.tile([128, HID], DT_MM, tag=f"w1{k}", name=f"w1{k}") for k in range(KT)]
            w2_t = [wp.tile([128, C], DT_MM, tag=f"w2{k}", name=f"w2{k}") for k in range(MH)]
            bias_t = wp.tile([128, 2 * KT + MH], F32, tag="bias")
            b2_t = wp.tile([1, C], DT_MM, tag="b2")
            eye_t = wp.tile([128, 128], DT_MM, tag="eye")
            ones_t = wp.tile([1, 128], DT_MM, tag="ones")
            # order matters: the SP DMA queue drains in program order, and
            # the first PE work needs eye (transposes) then wu/wg (phase B).
            nc.sync.dma_start(out=eye_t, in_=eye_d[:, :])
            nc.sync.dma_start(out=bias_t, in_=bias_d[:, :])

            def load_mid_weights():
                for k in range(KT):
                    s = slice(k * 128, (k + 1) * 128)
                    nc.sync.dma_start(out=wu_t[k], in_=wu_d[s, :])
                    nc.sync.dma_start(out=wg_t[k], in_=wg_d[s, :])
                for k in range(KT):
                    s = slice(k * 128, (k + 1) * 128)
                    nc.sync.dma_start(out=a_t[k], in_=a_d[s, :])
            def load_late_weights():
                # w1/w2 are first needed ~40us in; issuing them after the
                # first pair's x loads keeps the SP queue from delaying the
                # critical path.
                for k in range(KT):
                    s = slice(k * 128, (k + 1) * 128)
                    nc.sync.dma_start(out=w1_t[k], in_=w1_d[s, :])
                for k in range(MH):
                    nc.sync.dma_start(out=w2_t[k],
                                      in_=w2_d[k * 128:(k + 1) * 128, :])
                nc.sync.dma_start(out=b2_t, in_=b2_d[:, :])
                nc.sync.dma_start(out=ones_t, in_=ones_d[:, :])

            def newton_rstd(mvb, tag):
                """rstd[:, i] = rsqrt(var[:, i] + eps) for 4 batched tiles.

                Quadratic init + 3 Newton steps, all on the vector engine
                (no scalar-engine Sqrt -> no activation-table thrash).
                """
                v = mvb[:, :, 1:2].rearrange("p a b -> p (a b)")
                y = sp.tile([128, TPG], F32, tag=f"nsy{tag}", name=f"nsy{tag}")
                t = sp.tile([128, TPG], F32, tag=f"nst{tag}", name=f"nst{tag}")
                nc.vector.tensor_scalar(out=t, in0=v, scalar1=RS_C2,
                                        scalar2=RS_C1, op0=OP.mult, op1=OP.add)
                nc.vector.tensor_mul(out=y, in0=t, in1=v)
                nc.vector.tensor_scalar(out=y, in0=y, scalar1=RS_C0,
                                        scalar2=None, op0=OP.add)
                for _ in range(3):
                    nc.vector.tensor_mul(out=t, in0=y, in1=y)
                    nc.vector.scalar_tensor_tensor(out=t, in0=v, scalar=EPS,
                                                   in1=t, op0=OP.add,
                                                   op1=OP.mult)
                    nc.vector.tensor_scalar(out=t, in0=t, scalar1=-0.5,
                                            scalar2=1.5, op0=OP.mult,
                                            op1=OP.add)
                    nc.vector.tensor_mul(out=y, in0=y, in1=t)
                return y

            def phase_a_load(grp):
                """issue the x DMA loads early (prefetch)"""
                st = {}
                st["x_tm"] = x_tm = gp.tile([128, TPG, C], F32, tag="x_tm",
                                            name=f"x_tm{grp}", bufs=3)
                for it in range(TPG):
                    row0 = (grp * TPG + it) * 128
                    nc.sync.dma_start(out=x_tm[:, it, :],
                                      in_=x_in[row0:row0 + 128, :])
                return st

            def phase_a(grp, st):
                """norm1 + transpose -> channel-major xn"""
                x_tm = st["x_tm"]
                st["xn_cm"] = xn_cm = gp.tile([128, KT, GTOK], DT_MM,
                                              tag="xn_cm", name=f"xn_cm{grp}")
                mvb = sp.tile([128, TPG, 2], F32, tag="mvb", name=f"mvb{grp}")
                for it in range(TPG):
                    mv6 = sp.tile([128, 6], F32, tag="mv6")
                    nc.vector.bn_stats(out=mv6, in_=x_tm[:, it, :])
                    nc.vector.bn_aggr(out=mvb[:, it, :], in_=mv6)
                rstd = newton_rstd(mvb, "a")
                for it in range(TPG):
                    xn = tp.tile([128, C], DT_MM, tag="xn", bufs=2)
                    nc.vector.tensor_scalar(out=xn, in0=x_tm[:, it, :],
                                            scalar1=mvb[:, it, 0:1],
                                            scalar2=rstd[:, it:it + 1],
                                            op0=OP.subtract, op1=OP.mult)
                    pt = pst.tile([128, KT, 128], DT_MM, tag="pt")
                    for c in range(KT):
                        nc.tensor.transpose(pt[:, c, :],
                                            xn[:, c * 128:(c + 1) * 128],
                                            eye_t)
                    nc.scalar.copy(
                        out=xn_cm[:, :, it * 128:(it + 1) * 128], in_=pt)
                return st

            def phase_b(grp, st):
                """u/g projections, gate, m2"""
                xn_cm = st["xn_cm"]
                st["g"] = g_t = gp.tile([128, KT, GTOK], F32, tag="g",
                                        name=f"g{grp}")
                st["m2"] = m2_t = gp.tile([128, KT, GTOK], DT_MM, tag="m2",
                                          name=f"m2{grp}")
                psu = ps.tile([128, KT, GTOK], F32, tag="big", name=f"psu{grp}")
                psg = ps.tile([128, KT, GTOK], F32, tag="big", name=f"psg{grp}")
                for m in range(KT):
                    msl = slice(m * 128, (m + 1) * 128)
                    for k in range(KT):
                        nc.tensor.matmul(psu[:, m, :], wu_t[k][:, msl],
                                         xn_cm[:, k, :],
                                         start=(k == 0), stop=(k == KT - 1))
                    for k in range(KT):
                        nc.tensor.matmul(psg[:, m, :], wg_t[k][:, msl],
                                         xn_cm[:, k, :],
                                         start=(k == 0), stop=(k == KT - 1))
                for m in range(KT):
                    th = tp.tile([128, GTOK], F32, tag="th", bufs=2)
                    nc.scalar.activation(out=th, in_=psg[:, m, :], func=AF.Tanh,
                                         bias=bias_t[:, KT + m:KT + m + 1],
                                         scale=0.5)
                    nc.gpsimd.tensor_scalar(out=g_t[:, m, :], in0=th,
                                            scalar1=0.5, scalar2=0.5,
                                            op0=OP.mult, op1=OP.add)
                    ub = tp.tile([128, GTOK], DT_MM, tag="ub", bufs=2)
                    nc.scalar.activation(out=ub, in_=psu[:, m, :],
                                         func=AF.Identity,
                                         bias=bias_t[:, m:m + 1])
                    # m2 = (g - 1) * (u + bu)   (= -(1-g)*u_b)
                    nc.vector.scalar_tensor_tensor(
                        out=m2_t[:, m, :], in0=g_t[:, m, :], scalar=1.0,
                        in1=ub, op0=OP.subtract, op1=OP.mult)

            def scan_app(grp, st, src, tag):
                """dst = g * (A^T src), one gated application of B."""
                g_t = st["g"]
                dst = hp.tile([128, KT, GTOK], DT_MM, tag=tag,
                              name=f"{tag}{grp}")
                psb = ps.tile([128, KT, GTOK], F32, tag="big",
                              name=f"psb{grp}")
                for m in range(KT):
                    msl = slice(m * 128, (m + 1) * 128)
                    for k in range(KT):
                        nc.tensor.matmul(psb[:, m, :], a_t[k][:, msl],
                                         src[:, k, :],
                                         start=(k == 0), stop=(k == KT - 1))
                nc.vector.tensor_mul(out=dst, in0=g_t, in1=psb)
                return dst

            def residual1(grp, st):
                """x2 = x + h = x - h'  (token-major)"""
                h_t, x_tm = st["h"], st["x_tm"]
                st["x2_tm"] = x2_tm = gp.tile([128, TPG, C], F32, tag="x2_tm",
                                              name=f"x2_tm{grp}")
                for it in range(TPG):
                    pt = pst.tile([128, KT, 128], DT_MM, tag="pt")
                    for c in range(KT):
                        nc.tensor.transpose(
                            pt[:, c, :],
                            h_t[:, c, it * 128:(it + 1) * 128], eye_t)
                    nc.vector.tensor_sub(
                        out=x2_tm[:, it, :].rearrange("p (c q) -> p c q", c=KT),
                        in0=x_tm[:, it, :].rearrange("p (c q) -> p c q", c=KT),
                        in1=pt)

            def norm2(grp, st):
                x2_tm = st["x2_tm"]
                st["xn2_cm"] = xn2_cm = gp.tile([128, KT, GTOK], DT_MM,
                                                tag="xn2_cm",
                                                name=f"xn2_cm{grp}")
                mvb = sp.tile([128, TPG, 2], F32, tag="mvb2", name=f"mvb2{grp}")
                for it in range(TPG):
                    mv6 = sp.tile([128, 6], F32, tag="mv6")
                    nc.vector.bn_stats(out=mv6, in_=x2_tm[:, it, :])
                    nc.vector.bn_aggr(out=mvb[:, it, :], in_=mv6)
                rstd = newton_rstd(mvb, "b")
                for it in range(TPG):
                    xn2 = tp.tile([128, C], DT_MM, tag="xn", bufs=2)
                    nc.gpsimd.tensor_scalar(out=xn2, in0=x2_tm[:, it, :],
                                            scalar1=mvb[:, it, 0:1],
                                            scalar2=rstd[:, it:it + 1],
                                            op0=OP.subtract, op1=OP.mult)
                    pt = pst.tile([128, KT, 128], DT_MM, tag="pt")
                    for c in range(KT):
                        nc.tensor.transpose(pt[:, c, :],
                                            xn2[:, c * 128:(c + 1) * 128],
                                            eye_t)
                    nc.scalar.copy(
                        out=xn2_cm[:, :, it * 128:(it + 1) * 128], in_=pt)

            def mlp(grp, st):
                xn2_cm, x2_tm = st["xn2_cm"], st["x2_tm"]
                hid_t = hidp.tile([128, MH, GTOK], DT_MM, tag="hid",
                                  name=f"hid{grp}")
                for b in range(MH // KT):
                    psm = ps.tile([128, KT, GTOK], F32, tag="big",
                                  name=f"psm{grp}")
                    for m3 in range(KT):
                        mh = b * KT + m3
                        msl = slice(mh * 128, (mh + 1) * 128)
                        for k in range(KT):
                            nc.tensor.matmul(psm[:, m3, :], w1_t[k][:, msl],
                                             xn2_cm[:, k, :],
                                             start=(k == 0), stop=(k == KT - 1))
                    for m3 in range(KT):
                        mh = b * KT + m3
                        nc.scalar.activation(
                            out=hid_t[:, mh, :], in_=psm[:, m3, :],
                            func=AF.Gelu_apprx_tanh,
                            bias=bias_t[:, 2 * KT + mh:2 * KT + mh + 1],
                            scale=1.0)
                # second matmul: hidden is the stationary operand -> output
                # lands token-major; fold mlp_b2 in via a K=1 matmul.
                for it in range(TPG):
                    tsl = slice(it * 128, (it + 1) * 128)
                    pso = pst.tile([128, C], F32, tag="pt", name=f"pso{grp}")
                    for mh in range(MH):
                        nc.tensor.matmul(pso, hid_t[:, mh, tsl], w2_t[mh],
                                         start=(mh == 0), stop=False)
                    nc.tensor.matmul(pso, ones_t, b2_t,
                                     start=False, stop=True)
                    nc.vector.tensor_add(out=x2_tm[:, it, :],
                                         in0=x2_tm[:, it, :], in1=pso)
                    row0 = (grp * TPG + it) * 128
                    nc.sync.dma_start(out=out_d[row0:row0 + 128, :],
                                      in_=x2_tm[:, it, :])

            def scan(states, g0, g1):
                """h' = (I+B)(I+B^2)(I+B^4) m2', pairwise interleaved."""
                s0, s1 = states[g0], states[g1]
                cur = {g0: s0["m2"], g1: s1["m2"]}
                # b4 = B^4 m2
                for i in range(4):
                    cur[g0] = scan_app(g0, s0, cur[g0], "h")
                    cur[g1] = scan_app(g1, s1, cur[g1], "h")
                # r1 = m2 + b4
                r1 = {}
                for g, s in ((g0, s0), (g1, s1)):
                    r1[g] = hp.tile([128, KT, GTOK], DT_MM, tag="w",
                                    name=f"r1_{g}")
                    nc.vector.tensor_add(out=r1[g], in0=s["m2"], in1=cur[g])
                cur = dict(r1)
                for i in range(2):
                    cur[g0] = scan_app(g0, s0, cur[g0], "h")
                    cur[g1] = scan_app(g1, s1, cur[g1], "h")
                # r2 = r1 + c2
                r2 = {}
                for g in (g0, g1):
                    r2[g] = hp.tile([128, KT, GTOK], DT_MM, tag="w",
                                    name=f"r2_{g}")
                    nc.vector.tensor_add(out=r2[g], in0=r1[g], in1=cur[g])
                # d1 = B r2 ; h = r2 + d1
                d1 = {}
                d1[g0] = scan_app(g0, s0, r2[g0], "h")
                d1[g1] = scan_app(g1, s1, r2[g1], "h")
                for g, s in ((g0, s0), (g1, s1)):
                    h = hp.tile([128, KT, GTOK], DT_MM, tag="w", name=f"h_{g}")
                    nc.gpsimd.tensor_add(out=h, in0=r2[g], in1=d1[g])
                    s["h"] = h

            # Pairwise interleave groups so the PE fills one group's
            # scan/norm dependency stalls with the other group's matmuls;
            # additionally pipeline the next pair's phase A into the
            # current pair's norm2/MLP window.
            npair = (NG // 2) * repeat
            states = {}
            for pair_i in range(npair):
                pair = pair_i % (NG // 2)
                g0, g1 = 2 * pair, 2 * pair + 1
                if pair_i == 0:
                    states[g0] = phase_a_load(g0)
                    states[g1] = phase_a_load(g1)
                    phase_a(g0, states[g0])
                    phase_a(g1, states[g1])
                    load_mid_weights()
                s0, s1 = states[g0], states[g1]
                phase_b(g0, s0)
                phase_b(g1, s1)
                if pair_i == 0:
                    load_late_weights()
                scan(states, g0, g1)
                residual1(g0, s0)
                residual1(g1, s1)
                if pair_i + 1 < npair:
                    # issue next pair's x loads early so DMA overlaps
                    nx = 2 * ((pair_i + 1) % (NG // 2))
                    states[nx] = phase_a_load(nx)
                    states[nx + 1] = phase_a_load(nx + 1)
                norm2(g0, s0)
                norm2(g1, s1)
                mlp(g0, s0)
                mlp(g1, s1)
                if pair_i + 1 < npair:
                    # next pair's norm1/transposes AFTER this pair's mlp:
                    # keeps the scalar-engine FIFO and the pt-PSUM ring from
                    # blocking mlp2 behind next-pair work.
                    phase_a(nx, states[nx])
                    phase_a(nx + 1, states[nx + 1])
    return nc


_NC_CACHE = {}


def _get_nc():
    if "nc" not in _NC_CACHE:
        _NC_CACHE["nc"] = _patch_nc(build_nc())
    return _NC_CACHE["nc"]


# ---------------------------------------------------------------- kernel --
def kernel(x, norm1_scale, norm1_bias, Wu, bu, Wg, bg, A,
           norm2_scale, norm2_bias, mlp_w1, mlp_b1, mlp_w2, mlp_b2,
           _return_raw=False):
    f = np.float32
    x = np.asarray(x, f)
    norm1_scale = np.asarray(norm1_scale, f)
    norm1_bias = np.asarray(norm1_bias, f)
    Wu, bu = np.asarray(Wu, f), np.asarray(bu, f)
    Wg, bg = np.asarray(Wg, f), np.asarray(bg, f)
    A = np.asarray(A, f)
    norm2_scale = np.asarray(norm2_scale, f)
    norm2_bias = np.asarray(norm2_bias, f)
    mlp_w1, mlp_b1 = np.asarray(mlp_w1, f), np.asarray(mlp_b1, f)
    mlp_w2, mlp_b2 = np.asarray(mlp_w2, f), np.asarray(mlp_b2, f)

    # fold LN affine into downstream weights
    wu = np.ascontiguousarray(norm1_scale[:, None] * Wu)
    bu_f = bu + norm1_bias @ Wu
    wg = np.ascontiguousarray(norm1_scale[:, None] * Wg)
    bg_f = bg + norm1_bias @ Wg
    w1 = np.ascontiguousarray(norm2_scale[:, None] * mlp_w1)
    b1_f = mlp_b1 + norm2_bias @ mlp_w1

    # bias pack: [128, KT + KT + MH] columns = bu tiles, bg tiles, b1 tiles
    bias = np.empty((128, 2 * KT + MH), f)
    for m in range(KT):
        bias[:, m] = bu_f[m * 128:(m + 1) * 128]
        bias[:, KT + m] = 0.5 * bg_f[m * 128:(m + 1) * 128]
    for m in range(MH):
        bias[:, 2 * KT + m] = b1_f[m * 128:(m + 1) * 128]

    eye = np.eye(128, dtype=f)
    b2row = np.ascontiguousarray(mlp_b2[None, :])
    w2 = mlp_w2
    if DT_MM == BF16:
        import ml_dtypes
        bf = ml_dtypes.bfloat16
        wu, wg, w1 = wu.astype(bf), wg.astype(bf), w1.astype(bf)
        A = A.astype(bf)
        w2 = np.asarray(mlp_w2, f).astype(bf)
        b2row = b2row.astype(bf)
        eye = eye.astype(bf)
        ones = np.ones((1, 128), f).astype(bf)
    else:
        ones = np.ones((1, 128), f)

    xs = x.reshape(NCORES, NTOK, C)
    in_maps = [{
        "x": np.ascontiguousarray(xs[i]),
        "wu": wu, "wg": wg, "a": A, "w1": w1, "w2": w2,
        "bias": bias, "b2": b2row, "eye": eye, "ones": ones,
    } for i in range(NCORES)]

    res = run_bass_kernel_spmd(_get_nc(), in_maps, list(range(NCORES)))
    if _return_raw:
        return res
    out = np.concatenate([res.results[i]["out"] for i in range(NCORES)], axis=0)
    return out.reshape(B, H, W, C).astype(np.float32)
